# revision 58
# baseline (speedup 1.0000x reference)
"""DigitCapsuleLayer forward (2 routing iterations) on 8 Trainium2 cores.

Pure data-parallel: batch 256 split 32-per-core. Routing math restructured so
u_hat [B,2,6912,16] is never materialized:

  S[b,je]    = sum_m Wf[m,je] * x[m,b]          (m = (n,d) flattened, 55296)
  v1         = squash(0.5*S)
  g[m,b]     = sum_je Wf[m,je] * vtil[je,b]     (vtil = [v1_j0, -v1_j1])
  Delta[n,b] = sum_d g[(n,d),b] * x[(n,d),b]    (block-diag ones matmul)
  c0         = sigmoid(Delta) broadcast over d  (replication matmul)
  y0         = c0 * x
  A[b,je]    = sum_m Wf[m,je] * y0[m,b]
  s2_j0 = A_j0 ; s2_j1 = S_j1 - A_j1            (since c1 = 1-c0)
  v = squash(s2)

Perf structure:
 - phase-1 S and phase-6 A matmuls are 4-tile packed ([128x128]x[128x128]
   with diagonal-block extraction) -> 108 matmuls instead of 432 each,
   cutting PE sequencer time 4x.
 - DMA order: xt/wf slices interleaved (phase-1 streams during DMA),
   then wft (fp8, only needed once the routing pipeline starts).
 - PSUM->SBUF g copies run mostly on the (otherwise idle) GpSimd engine,
   sigmoid on Activation, multiplies on Vector.
"""

import os
os.environ.setdefault("NEURON_RT_RESET_CORES", "1")

import numpy as np
import ml_dtypes

import concourse.bacc as bacc
import concourse.mybir as mybir
import concourse.tile as tile
from concourse.bass_utils import run_bass_kernel_spmd

# Problem constants (hardcoded per harness contract)
B = 256
NCORES = 8
BC = B // NCORES          # 32 batch per core
BG = 16                   # batch per group (2 groups per core)
N = 6912
D = 8
E = 16
J = 2
M = N * D                 # 55296
JE = J * E                # 32
NT = M // 128             # 432 m-tiles
NP = NT // 4              # 108 4-tile packs
NG = NT // 4              # 108 groups of 4 (row-packed g matmuls)
GH = NT * 16              # 6912 cols per group monolith
NCH = 14                  # 32-tile chunks per group (13 full + 1 half)
FREE = NT * BC            # 13824
EPS = 1e-9

BF16 = mybir.dt.bfloat16
F8 = mybir.dt.float8e4
F32 = mybir.dt.float32

_cached = None


def _build_program():
    nc = bacc.Bacc("TRN2", num_devices=NCORES)

    xt = nc.dram_tensor("xt", [128, FREE], BF16, kind="ExternalInput")
    wf = nc.dram_tensor("wf", [128, FREE], BF16, kind="ExternalInput")
    wft = nc.dram_tensor("wft", [128, NG * 128], F8, kind="ExternalInput")
    sumrep = nc.dram_tensor("sumrep", [128, 128], BF16, kind="ExternalInput")
    vout = nc.dram_tensor("vout", [BC, JE], F32, kind="ExternalOutput")

    SIG = mybir.ActivationFunctionType.Sigmoid
    SQRT = mybir.ActivationFunctionType.Sqrt

    with tile.TileContext(nc) as tc:
        with (
            tc.tile_pool(name="big", bufs=1) as big,
            tc.tile_pool(name="small", bufs=1) as small,
            tc.tile_pool(name="p_gbf", bufs=3) as p_gbf,
            tc.tile_pool(name="p_tch", bufs=4) as p_tch,
            tc.tile_pool(name="p_cbf", bufs=4) as p_cbf,
            tc.tile_pool(name="p_ybf", bufs=5) as p_ybf,
            tc.tile_pool(name="ps_S", bufs=1, space="PSUM") as ps_S,
            tc.tile_pool(name="ps_A", bufs=1, space="PSUM") as ps_A,
            tc.tile_pool(name="ps_g", bufs=4, space="PSUM") as ps_g,
            tc.tile_pool(name="ps_d", bufs=2, space="PSUM") as ps_d,
        ):
            XT = big.tile([128, FREE], BF16, tag="XT")
            WF = big.tile([128, FREE], BF16, tag="WF")
            WFT = big.tile([128, NG * 128], F8, tag="WFT")
            YB = big.tile([128, FREE], BF16, tag="YB")   # y0, (t, 32b) layout
            SUMREP = small.tile([128, 128], BF16, tag="SUMREP")
            VTBD0 = small.tile([128, 64], BF16, tag="VTBD0")
            VTBD1 = small.tile([128, 64], BF16, tag="VTBD1")
            VTBD = (VTBD0, VTBD1)
            nc.vector.memset(VTBD0[:], 0.0)
            nc.vector.memset(VTBD1[:], 0.0)

            # ---- DMA: sumrep first, then interleaved xt/wf slices; wft last ----
            nc.sync.dma_start(SUMREP[:], sumrep[:])
            wsl = FREE // 8
            xsl = GH // 4
            for i in range(4):
                nc.sync.dma_start(WF[:, 2 * i * wsl:(2 * i + 1) * wsl],
                                  wf[:, 2 * i * wsl:(2 * i + 1) * wsl])
                nc.sync.dma_start(WF[:, (2 * i + 1) * wsl:(2 * i + 2) * wsl],
                                  wf[:, (2 * i + 1) * wsl:(2 * i + 2) * wsl])
                nc.sync.dma_start(XT[:, i * xsl:(i + 1) * xsl],
                                  xt[:, i * xsl:(i + 1) * xsl])
            tsl = NG * 128 // 8
            for i in range(8):
                nc.sync.dma_start(WFT[:, i * tsl:(i + 1) * tsl],
                                  wft[:, i * tsl:(i + 1) * tsl])
            for i in range(4):
                nc.sync.dma_start(XT[:, GH + i * xsl:GH + (i + 1) * xsl],
                                  xt[:, GH + i * xsl:GH + (i + 1) * xsl])

            # S and A accumulate TRANSPOSED ([32 je, b]) so extraction is
            # a transpose and all batch slicing is free-dim
            psT0 = ps_S.tile([32, 32], F32, tag="psT0")
            psT1 = ps_A.tile([32, 32], F32, tag="psT1")

            # PE p-state warmers: tiny matmuls that keep the tensor engine
            # continuously busy so it ramps to (and stays at) full clock.
            # Scratch target borrows a ps_d buffer (pipeline reuses it later,
            # after all warmers are done).
            wtile = ps_d.tile([128, 512], F32, tag="psd")

            def warm(n):
                for _ in range(n):
                    nc.tensor.matmul(
                        wtile[:, 0:64], lhsT=SUMREP[:], rhs=SUMREP[:, 0:64],
                        start=True, stop=True,
                    )
            def s_mm(g, t):
                ps = psT0 if g == 0 else psT1
                nc.tensor.matmul(
                    ps[:, 0:BG],
                    lhsT=WF[:, t * JE:(t + 1) * JE],
                    rhs=XT[:, g * GH + t * BG:g * GH + (t + 1) * BG],
                    start=(t == 0),
                    stop=(t == NT - 1),
                )

            warm(60)  # bridge the wait for the first wf/x slices
            for t in range(NT):
                s_mm(0, t)
                if t % 3 == 0:
                    warm(2)
            warm(40)
            ST = small.tile([JE, BC], F32, tag="ST")   # transposed S, both b

            SP0 = small.tile([32, 32], F32, tag="SP0")
            SP1 = small.tile([32, 32], F32, tag="SP1")
            vtpad = small.tile([32, 32], BF16, tag="vtpad")
            vtT = small.tile([32, 32], BF16, tag="vtT")
            nc.vector.memset(vtpad[:], 0.0)
            MUL = mybir.AluOpType.mult
            ADD = mybir.AluOpType.add

            def squash1(g):
                ps = psT0 if g == 0 else psT1
                SP = SP0 if g == 0 else SP1
                nc.vector.tensor_copy(ST[:, BG * g:BG * g + BG], ps[:, 0:BG])
                nc.vector.transpose(SP[:], ps[:])     # rows 0:16 = S_g[b,je]
                Sg = SP[0:BG, :]
                sq = small.tile([BG, JE], F32, tag=f"sq{g}")
                n2s = small.tile([BG, J], F32, tag=f"n2s{g}")
                d1 = small.tile([BG, J], F32, tag=f"d1{g}")
                r1 = small.tile([BG, J], F32, tag=f"r1{g}")
                q = small.tile([BG, J], F32, tag=f"q{g}")
                rq = small.tile([BG, J], F32, tag=f"rq{g}")
                f = small.tile([BG, J], F32, tag=f"f{g}")
                nc.vector.tensor_mul(sq[:], Sg, Sg)
                nc.vector.reduce_sum(
                    n2s[:], sq.rearrange("p (j e) -> p j e", e=E),
                    axis=mybir.AxisListType.X)
                nc.vector.tensor_scalar(d1[:], n2s[:], 0.25, 1.0, MUL, ADD)
                nc.vector.reciprocal(r1[:], d1[:])
                nc.vector.tensor_scalar(q[:], n2s[:], 0.25, EPS, MUL, ADD)
                nc.scalar.activation(q[:], q[:], SQRT)
                nc.vector.reciprocal(rq[:], q[:])
                nc.vector.tensor_mul(f[:], r1[:], rq[:])
                nc.vector.tensor_mul(f[:], f[:], n2s[:])
                nc.vector.tensor_scalar(
                    vtpad[0:BG, 0:E], Sg[:, 0:E], f[:, 0:1], 0.125, MUL, MUL)
                nc.vector.tensor_scalar(
                    vtpad[0:BG, E:JE], Sg[:, E:JE], f[:, 1:2], -0.125, MUL, MUL)
                nc.vector.transpose(vtT[:], vtpad[:])
                for a in range(4):
                    nc.vector.tensor_copy(
                        VTBD[g][32 * a:32 * a + 32, 16 * a:16 * a + 16],
                        vtT[:, 0:BG])

            squash1(0)
            psAT = ps_S.tile([32, 32], F32, tag="psT0")  # ring-reuse psT0

            # ---- merged pipeline over 28 chunks (14 per group) ----
            psg_l, tch_l, psd_l, cbf_l = {}, {}, {}, {}
            NC2 = 2 * NCH

            def cw(K):
                return 512 if K % NCH < NCH - 1 else 256   # last chunk half

            def gk(c):
                return c // NCH, c % NCH

            def g_stage(c):
                g, K = gk(c)
                psg = ps_g.tile([128, 512], F32, tag="psg")
                for qq in range(cw(c) // 64):
                    blk = 8 * K + qq
                    nc.tensor.matmul(
                        psg[:, qq * 64:(qq + 1) * 64],
                        lhsT=WFT[:, blk * 128:(blk + 1) * 128],
                        rhs=VTBD[g][:],
                        start=True, stop=True,
                    )
                psg_l[c] = psg

            def t_stage(c):
                g, K = gk(c)
                w = cw(c)
                lo = g * GH + K * 512
                tch = p_tch.tile([128, 512], BF16, tag="tch")
                nc.vector.tensor_mul(
                    tch[:, 0:w], psg_l.pop(c)[:, 0:w], XT[:, lo:lo + w])
                tch_l[c] = tch

            def d_stage(c):
                w = cw(c)
                psd = ps_d.tile([128, 512], F32, tag="psd")
                nc.tensor.matmul(
                    psd[:, 0:w], lhsT=SUMREP[:], rhs=tch_l.pop(c)[:, 0:w],
                    start=True, stop=True)
                psd_l[c] = psd

            def sig_stage(c):
                w = cw(c)
                cbf = p_cbf.tile([128, 512], BF16, tag="cbf")
                nc.scalar.activation(cbf[:, 0:w], psd_l.pop(c)[:, 0:w], SIG)
                cbf_l[c] = cbf

            def y_stage(c):
                g, K = gk(c)
                w = cw(c)
                tw = w // BG
                lo = g * GH + K * 512
                dst = YB.rearrange("p (t b) -> p t b", b=BC)[
                    :, 32 * K:32 * K + tw, BG * g:BG * g + BG]
                eng = nc.gpsimd if (c % 2 == 0 and c < NC2 - 2) else nc.vector
                eng.tensor_mul(
                    dst,
                    cbf_l.pop(c).rearrange("p (t b) -> p t b", b=BG)[:, 0:tw, :],
                    XT[:, lo:lo + w].rearrange("p (t b) -> p t b", b=BG),
                )

            def s2_stage(c):
                # only group-1 chunks trigger A matmuls (cover both groups)
                g, K = gk(c)
                if g == 0:
                    return
                for ii in range(cw(c) // BG):
                    t = 32 * K + ii
                    nc.tensor.matmul(
                        psAT[:],
                        lhsT=WF[:, t * JE:(t + 1) * JE],
                        rhs=YB[:, t * BC:(t + 1) * BC],
                        start=(t == 0),
                        stop=(t == NT - 1),
                    )

            for i in range(NC2 + 10):
                if i < NC2:
                    g_stage(i)
                if 4 <= i <= 10:
                    for t in range(62 * (i - 4), min(62 * (i - 3), NT)):
                        s_mm(1, t)
                if i == 10:
                    squash1(1)
                if 0 <= i - 2 < NC2:
                    t_stage(i - 2)
                if 0 <= i - 4 < NC2:
                    d_stage(i - 4)
                if 0 <= i - 5 < NC2:
                    sig_stage(i - 5)
                if 0 <= i - 6 < NC2:
                    y_stage(i - 6)
                if 0 <= i - 9 < NC2:
                    s2_stage(i - 9)

            # ---- A/S transposes + s2 combine + squash #2 ----
            A = small.tile([BC, JE], F32, tag="A")
            S = small.tile([BC, JE], F32, tag="S")
            nc.vector.transpose(A[:], psAT[:])
            nc.vector.transpose(S[:], ST[:])

            s2 = small.tile([BC, JE], F32, tag="s2")
            sq2 = small.tile([BC, JE], F32, tag="sq2")
            n2b = small.tile([BC, J], F32, tag="n2b")
            d1b = small.tile([BC, J], F32, tag="d1b")
            r1b = small.tile([BC, J], F32, tag="r1b")
            qb = small.tile([BC, J], F32, tag="qb")
            rqb = small.tile([BC, J], F32, tag="rqb")
            fb = small.tile([BC, J], F32, tag="fb")
            v2 = small.tile([BC, JE], F32, tag="v2")

            nc.vector.tensor_copy(s2[:, 0:E], A[:, 0:E])
            nc.vector.tensor_sub(s2[:, E:JE], S[:, E:JE], A[:, E:JE])
            nc.vector.tensor_mul(sq2[:], s2[:], s2[:])
            nc.vector.reduce_sum(
                n2b[:], sq2.rearrange("p (j e) -> p j e", e=E), axis=mybir.AxisListType.X
            )
            nc.vector.tensor_scalar(d1b[:], n2b[:], 1.0, None, ADD)
            nc.vector.reciprocal(r1b[:], d1b[:])
            nc.vector.tensor_scalar(qb[:], n2b[:], EPS, None, ADD)
            nc.scalar.activation(qb[:], qb[:], SQRT)
            nc.vector.reciprocal(rqb[:], qb[:])
            nc.vector.tensor_mul(fb[:], r1b[:], rqb[:])
            nc.vector.tensor_scalar(v2[:, 0:E], s2[:, 0:E], fb[:, 0:1], n2b[:, 0:1], MUL, MUL)
            nc.vector.tensor_scalar(v2[:, E:JE], s2[:, E:JE], fb[:, 1:2], n2b[:, 1:2], MUL, MUL)

            nc.sync.dma_start(vout[:], v2[:])

    nc.compile()
    return nc


def _prep_host(x, W):
    """Build per-core DRAM feeds (identical layouts to prior version)."""
    bf = ml_dtypes.bfloat16
    # Wf[(n,d), (j,e)] = W[j,n,e,d]
    Wf = np.ascontiguousarray(np.transpose(W, (1, 3, 0, 2)).reshape(M, JE))
    wf_feed = np.ascontiguousarray(
        Wf.reshape(NT, 128, JE).transpose(1, 0, 2).reshape(128, FREE)
    ).astype(bf)
    # 4-stacked WfT groups: group g rows 32a+k hold Wf[m=128*(4g+a)+f, k]
    wft_np = np.empty((NG, 128, 128), dtype=np.float32)
    blocks = Wf.reshape(NT, 128, JE)                    # [432, 128, 32]
    for a in range(4):
        wft_np[:, 32 * a:32 * a + 32, :] = blocks[a::4].transpose(0, 2, 1)
    wft_feed = np.ascontiguousarray(
        wft_np.transpose(1, 0, 2).reshape(128, NG * 128)
    ).astype(ml_dtypes.float8_e4m3)

    p = np.arange(128)
    sumrep_np = (p[:, None] // D == p[None, :] // D).astype(bf)

    in_maps = []
    for c in range(NCORES):
        xs = x[c * BC:(c + 1) * BC].reshape(BC, M).T      # [m, 32b]
        halves = []
        for g in range(2):
            xg = xs[:, 16 * g:16 * g + 16]                # [m, 16b]
            halves.append(np.ascontiguousarray(
                xg.reshape(NT, 128, 16).transpose(1, 0, 2).reshape(128, GH)))
        xt_feed = np.ascontiguousarray(np.concatenate(halves, axis=1)).astype(bf)
        in_maps.append({
            "xt": xt_feed,
            "wf": wf_feed,
            "wft": wft_feed,
            "sumrep": sumrep_np,
        })
    return in_maps


def kernel(x, W):
    global _cached
    x = np.asarray(x, dtype=np.float32)
    W = np.asarray(W, dtype=np.float32)
    if _cached is None:
        _cached = _build_program()
    nc = _cached
    in_maps = _prep_host(x, W)
    res = run_bass_kernel_spmd(nc, in_maps, list(range(NCORES)))
    out = np.concatenate(
        [res.results[c]["vout"].reshape(BC, J, E) for c in range(NCORES)], axis=0
    )
    return out.astype(np.float32)


if __name__ == "__main__":
    import sys
    sys.path.insert(0, "/root/problem")
    import reference as ref
    inputs = ref.setup_inputs()
    expected = np.asarray(ref.reference(**inputs))
    actual = kernel(np.asarray(inputs["x"]), np.asarray(inputs["W"]))
    err = np.abs(actual - expected)
    scale = np.abs(expected).max()
    print("absmax err:", err.max(), "scale:", scale, "rel:", err.max() / scale)


# revision 59
# speedup vs baseline: 1.0030x; 1.0030x over previous
"""DigitCapsuleLayer forward (2 routing iterations) on 8 Trainium2 cores.

Pure data-parallel: batch 256 split 32-per-core. Routing math restructured so
u_hat [B,2,6912,16] is never materialized:

  S[b,je]    = sum_m Wf[m,je] * x[m,b]          (m = (n,d) flattened, 55296)
  v1         = squash(0.5*S)
  g[m,b]     = sum_je Wf[m,je] * vtil[je,b]     (vtil = [v1_j0, -v1_j1])
  Delta[n,b] = sum_d g[(n,d),b] * x[(n,d),b]    (block-diag ones matmul)
  c0         = sigmoid(Delta) broadcast over d  (replication matmul)
  y0         = c0 * x
  A[b,je]    = sum_m Wf[m,je] * y0[m,b]
  s2_j0 = A_j0 ; s2_j1 = S_j1 - A_j1            (since c1 = 1-c0)
  v = squash(s2)

Perf structure:
 - phase-1 S and phase-6 A matmuls are 4-tile packed ([128x128]x[128x128]
   with diagonal-block extraction) -> 108 matmuls instead of 432 each,
   cutting PE sequencer time 4x.
 - DMA order: xt/wf slices interleaved (phase-1 streams during DMA),
   then wft (fp8, only needed once the routing pipeline starts).
 - PSUM->SBUF g copies run mostly on the (otherwise idle) GpSimd engine,
   sigmoid on Activation, multiplies on Vector.
"""

import os
os.environ.setdefault("NEURON_RT_RESET_CORES", "1")

import numpy as np
import ml_dtypes

import concourse.bacc as bacc
import concourse.mybir as mybir
import concourse.tile as tile
from concourse.bass_utils import run_bass_kernel_spmd

# Problem constants (hardcoded per harness contract)
B = 256
NCORES = 8
BC = B // NCORES          # 32 batch per core
BG = 16                   # batch per group (2 groups per core)
N = 6912
D = 8
E = 16
J = 2
M = N * D                 # 55296
JE = J * E                # 32
NT = M // 128             # 432 m-tiles
NP = NT // 4              # 108 4-tile packs
NG = NT // 4              # 108 groups of 4 (row-packed g matmuls)
GH = NT * 16              # 6912 cols per group monolith
NCH = 14                  # 32-tile chunks per group (13 full + 1 half)
FREE = NT * BC            # 13824
EPS = 1e-9

BF16 = mybir.dt.bfloat16
F8 = mybir.dt.float8e4
F32 = mybir.dt.float32

_cached = None


def _build_program():
    nc = bacc.Bacc("TRN2", num_devices=NCORES)

    xt = nc.dram_tensor("xt", [128, FREE], BF16, kind="ExternalInput")
    wf = nc.dram_tensor("wf", [128, FREE], BF16, kind="ExternalInput")
    wft = nc.dram_tensor("wft", [128, NG * 128], F8, kind="ExternalInput")
    sumrep = nc.dram_tensor("sumrep", [128, 128], BF16, kind="ExternalInput")
    vout = nc.dram_tensor("vout", [BC, JE], F32, kind="ExternalOutput")

    SIG = mybir.ActivationFunctionType.Sigmoid
    SQRT = mybir.ActivationFunctionType.Sqrt

    with tile.TileContext(nc) as tc:
        with (
            tc.tile_pool(name="big", bufs=1) as big,
            tc.tile_pool(name="small", bufs=1) as small,
            tc.tile_pool(name="p_gbf", bufs=3) as p_gbf,
            tc.tile_pool(name="p_tch", bufs=5) as p_tch,
            tc.tile_pool(name="p_cbf", bufs=5) as p_cbf,
            tc.tile_pool(name="p_ybf", bufs=5) as p_ybf,
            tc.tile_pool(name="ps_S", bufs=1, space="PSUM") as ps_S,
            tc.tile_pool(name="ps_A", bufs=1, space="PSUM") as ps_A,
            tc.tile_pool(name="ps_g", bufs=4, space="PSUM") as ps_g,
            tc.tile_pool(name="ps_d", bufs=2, space="PSUM") as ps_d,
        ):
            XT = big.tile([128, FREE], BF16, tag="XT")
            WF = big.tile([128, FREE], BF16, tag="WF")
            WFT = big.tile([128, NG * 128], F8, tag="WFT")
            YB = big.tile([128, FREE], BF16, tag="YB")   # y0, (t, 32b) layout
            SUMREP = small.tile([128, 128], BF16, tag="SUMREP")
            VTBD0 = small.tile([128, 64], BF16, tag="VTBD0")
            VTBD1 = small.tile([128, 64], BF16, tag="VTBD1")
            VTBD = (VTBD0, VTBD1)
            nc.vector.memset(VTBD0[:], 0.0)
            nc.vector.memset(VTBD1[:], 0.0)

            # ---- DMA: sumrep first, then interleaved xt/wf slices; wft last ----
            nc.sync.dma_start(SUMREP[:], sumrep[:])
            wsl = FREE // 8
            xsl = GH // 4
            for i in range(4):
                nc.sync.dma_start(WF[:, 2 * i * wsl:(2 * i + 1) * wsl],
                                  wf[:, 2 * i * wsl:(2 * i + 1) * wsl])
                nc.sync.dma_start(WF[:, (2 * i + 1) * wsl:(2 * i + 2) * wsl],
                                  wf[:, (2 * i + 1) * wsl:(2 * i + 2) * wsl])
                nc.sync.dma_start(XT[:, i * xsl:(i + 1) * xsl],
                                  xt[:, i * xsl:(i + 1) * xsl])
            tsl = NG * 128 // 8
            for i in range(8):
                nc.sync.dma_start(WFT[:, i * tsl:(i + 1) * tsl],
                                  wft[:, i * tsl:(i + 1) * tsl])
            for i in range(4):
                nc.sync.dma_start(XT[:, GH + i * xsl:GH + (i + 1) * xsl],
                                  xt[:, GH + i * xsl:GH + (i + 1) * xsl])

            # S and A accumulate TRANSPOSED ([32 je, b]) so extraction is
            # a transpose and all batch slicing is free-dim
            psT0 = ps_S.tile([32, 32], F32, tag="psT0")
            psT1 = ps_A.tile([32, 32], F32, tag="psT1")

            # PE p-state warmers: tiny matmuls that keep the tensor engine
            # continuously busy so it ramps to (and stays at) full clock.
            # Scratch target borrows a ps_d buffer (pipeline reuses it later,
            # after all warmers are done).
            wtile = ps_d.tile([128, 512], F32, tag="psd")

            def warm(n):
                for _ in range(n):
                    nc.tensor.matmul(
                        wtile[:, 0:64], lhsT=SUMREP[:], rhs=SUMREP[:, 0:64],
                        start=True, stop=True,
                    )
            def s_mm(g, t):
                ps = psT0 if g == 0 else psT1
                nc.tensor.matmul(
                    ps[:, 0:BG],
                    lhsT=WF[:, t * JE:(t + 1) * JE],
                    rhs=XT[:, g * GH + t * BG:g * GH + (t + 1) * BG],
                    start=(t == 0),
                    stop=(t == NT - 1),
                )

            warm(60)  # bridge the wait for the first wf/x slices
            for t in range(NT):
                s_mm(0, t)
                if t % 3 == 0:
                    warm(2)
            warm(40)
            ST = small.tile([JE, BC], F32, tag="ST")   # transposed S, both b

            SP0 = small.tile([32, 32], F32, tag="SP0")
            SP1 = small.tile([32, 32], F32, tag="SP1")
            vtpad = small.tile([32, 32], BF16, tag="vtpad")
            vtT = small.tile([32, 32], BF16, tag="vtT")
            nc.vector.memset(vtpad[:], 0.0)
            MUL = mybir.AluOpType.mult
            ADD = mybir.AluOpType.add

            def squash1(g):
                ps = psT0 if g == 0 else psT1
                SP = SP0 if g == 0 else SP1
                nc.vector.tensor_copy(ST[:, BG * g:BG * g + BG], ps[:, 0:BG])
                nc.vector.transpose(SP[:], ps[:])     # rows 0:16 = S_g[b,je]
                Sg = SP[0:BG, :]
                sq = small.tile([BG, JE], F32, tag=f"sq{g}")
                n2s = small.tile([BG, J], F32, tag=f"n2s{g}")
                d1 = small.tile([BG, J], F32, tag=f"d1{g}")
                r1 = small.tile([BG, J], F32, tag=f"r1{g}")
                q = small.tile([BG, J], F32, tag=f"q{g}")
                rq = small.tile([BG, J], F32, tag=f"rq{g}")
                f = small.tile([BG, J], F32, tag=f"f{g}")
                nc.vector.tensor_mul(sq[:], Sg, Sg)
                nc.vector.reduce_sum(
                    n2s[:], sq.rearrange("p (j e) -> p j e", e=E),
                    axis=mybir.AxisListType.X)
                nc.vector.tensor_scalar(d1[:], n2s[:], 0.25, 1.0, MUL, ADD)
                nc.vector.reciprocal(r1[:], d1[:])
                nc.vector.tensor_scalar(q[:], n2s[:], 0.25, EPS, MUL, ADD)
                nc.scalar.activation(q[:], q[:], SQRT)
                nc.vector.reciprocal(rq[:], q[:])
                nc.vector.tensor_mul(f[:], r1[:], rq[:])
                nc.vector.tensor_mul(f[:], f[:], n2s[:])
                nc.vector.tensor_scalar(
                    vtpad[0:BG, 0:E], Sg[:, 0:E], f[:, 0:1], 0.125, MUL, MUL)
                nc.vector.tensor_scalar(
                    vtpad[0:BG, E:JE], Sg[:, E:JE], f[:, 1:2], -0.125, MUL, MUL)
                nc.vector.transpose(vtT[:], vtpad[:])
                for a in range(4):
                    nc.vector.tensor_copy(
                        VTBD[g][32 * a:32 * a + 32, 16 * a:16 * a + 16],
                        vtT[:, 0:BG])

            squash1(0)
            psAT = ps_S.tile([32, 32], F32, tag="psT0")  # ring-reuse psT0

            # ---- merged pipeline over 28 chunks (14 per group) ----
            psg_l, tch_l, psd_l, cbf_l = {}, {}, {}, {}
            NC2 = 2 * NCH

            def cw(K):
                return 512 if K % NCH < NCH - 1 else 256   # last chunk half

            def gk(c):
                return c // NCH, c % NCH

            def g_stage(c):
                g, K = gk(c)
                psg = ps_g.tile([128, 512], F32, tag="psg")
                for qq in range(cw(c) // 64):
                    blk = 8 * K + qq
                    nc.tensor.matmul(
                        psg[:, qq * 64:(qq + 1) * 64],
                        lhsT=WFT[:, blk * 128:(blk + 1) * 128],
                        rhs=VTBD[g][:],
                        start=True, stop=True,
                    )
                psg_l[c] = psg

            def t_stage(c):
                g, K = gk(c)
                w = cw(c)
                lo = g * GH + K * 512
                tch = p_tch.tile([128, 512], BF16, tag="tch")
                nc.vector.tensor_mul(
                    tch[:, 0:w], psg_l.pop(c)[:, 0:w], XT[:, lo:lo + w])
                tch_l[c] = tch

            def d_stage(c):
                w = cw(c)
                psd = ps_d.tile([128, 512], F32, tag="psd")
                nc.tensor.matmul(
                    psd[:, 0:w], lhsT=SUMREP[:], rhs=tch_l.pop(c)[:, 0:w],
                    start=True, stop=True)
                psd_l[c] = psd

            def sig_stage(c):
                w = cw(c)
                cbf = p_cbf.tile([128, 512], BF16, tag="cbf")
                nc.scalar.activation(cbf[:, 0:w], psd_l.pop(c)[:, 0:w], SIG)
                cbf_l[c] = cbf

            def y_stage(c):
                g, K = gk(c)
                w = cw(c)
                tw = w // BG
                lo = g * GH + K * 512
                dst = YB.rearrange("p (t b) -> p t b", b=BC)[
                    :, 32 * K:32 * K + tw, BG * g:BG * g + BG]
                eng = nc.gpsimd if (c % 2 == 0 and c < NC2 - 2) else nc.vector
                eng.tensor_mul(
                    dst,
                    cbf_l.pop(c).rearrange("p (t b) -> p t b", b=BG)[:, 0:tw, :],
                    XT[:, lo:lo + w].rearrange("p (t b) -> p t b", b=BG),
                )

            def s2_stage(c):
                # only group-1 chunks trigger A matmuls (cover both groups)
                g, K = gk(c)
                if g == 0:
                    return
                for ii in range(cw(c) // BG):
                    t = 32 * K + ii
                    nc.tensor.matmul(
                        psAT[:],
                        lhsT=WF[:, t * JE:(t + 1) * JE],
                        rhs=YB[:, t * BC:(t + 1) * BC],
                        start=(t == 0),
                        stop=(t == NT - 1),
                    )

            for i in range(NC2 + 10):
                if i < NC2:
                    g_stage(i)
                if 4 <= i <= 10:
                    for t in range(62 * (i - 4), min(62 * (i - 3), NT)):
                        s_mm(1, t)
                if i == 10:
                    squash1(1)
                if 0 <= i - 2 < NC2:
                    t_stage(i - 2)
                if 0 <= i - 4 < NC2:
                    d_stage(i - 4)
                if 0 <= i - 5 < NC2:
                    sig_stage(i - 5)
                if 0 <= i - 6 < NC2:
                    y_stage(i - 6)
                if 0 <= i - 9 < NC2:
                    s2_stage(i - 9)

            # ---- A/S transposes + s2 combine + squash #2 ----
            A = small.tile([BC, JE], F32, tag="A")
            S = small.tile([BC, JE], F32, tag="S")
            nc.vector.transpose(A[:], psAT[:])
            nc.vector.transpose(S[:], ST[:])

            s2 = small.tile([BC, JE], F32, tag="s2")
            sq2 = small.tile([BC, JE], F32, tag="sq2")
            n2b = small.tile([BC, J], F32, tag="n2b")
            d1b = small.tile([BC, J], F32, tag="d1b")
            r1b = small.tile([BC, J], F32, tag="r1b")
            qb = small.tile([BC, J], F32, tag="qb")
            rqb = small.tile([BC, J], F32, tag="rqb")
            fb = small.tile([BC, J], F32, tag="fb")
            v2 = small.tile([BC, JE], F32, tag="v2")

            nc.vector.tensor_copy(s2[:, 0:E], A[:, 0:E])
            nc.vector.tensor_sub(s2[:, E:JE], S[:, E:JE], A[:, E:JE])
            nc.vector.tensor_mul(sq2[:], s2[:], s2[:])
            nc.vector.reduce_sum(
                n2b[:], sq2.rearrange("p (j e) -> p j e", e=E), axis=mybir.AxisListType.X
            )
            nc.vector.tensor_scalar(d1b[:], n2b[:], 1.0, None, ADD)
            nc.vector.reciprocal(r1b[:], d1b[:])
            nc.vector.tensor_scalar(qb[:], n2b[:], EPS, None, ADD)
            nc.scalar.activation(qb[:], qb[:], SQRT)
            nc.vector.reciprocal(rqb[:], qb[:])
            nc.vector.tensor_mul(fb[:], r1b[:], rqb[:])
            nc.vector.tensor_scalar(v2[:, 0:E], s2[:, 0:E], fb[:, 0:1], n2b[:, 0:1], MUL, MUL)
            nc.vector.tensor_scalar(v2[:, E:JE], s2[:, E:JE], fb[:, 1:2], n2b[:, 1:2], MUL, MUL)

            nc.sync.dma_start(vout[:], v2[:])

    nc.compile()
    return nc


def _prep_host(x, W):
    """Build per-core DRAM feeds (identical layouts to prior version)."""
    bf = ml_dtypes.bfloat16
    # Wf[(n,d), (j,e)] = W[j,n,e,d]
    Wf = np.ascontiguousarray(np.transpose(W, (1, 3, 0, 2)).reshape(M, JE))
    wf_feed = np.ascontiguousarray(
        Wf.reshape(NT, 128, JE).transpose(1, 0, 2).reshape(128, FREE)
    ).astype(bf)
    # 4-stacked WfT groups: group g rows 32a+k hold Wf[m=128*(4g+a)+f, k]
    wft_np = np.empty((NG, 128, 128), dtype=np.float32)
    blocks = Wf.reshape(NT, 128, JE)                    # [432, 128, 32]
    for a in range(4):
        wft_np[:, 32 * a:32 * a + 32, :] = blocks[a::4].transpose(0, 2, 1)
    wft_feed = np.ascontiguousarray(
        wft_np.transpose(1, 0, 2).reshape(128, NG * 128)
    ).astype(ml_dtypes.float8_e4m3)

    p = np.arange(128)
    sumrep_np = (p[:, None] // D == p[None, :] // D).astype(bf)

    in_maps = []
    for c in range(NCORES):
        xs = x[c * BC:(c + 1) * BC].reshape(BC, M).T      # [m, 32b]
        halves = []
        for g in range(2):
            xg = xs[:, 16 * g:16 * g + 16]                # [m, 16b]
            halves.append(np.ascontiguousarray(
                xg.reshape(NT, 128, 16).transpose(1, 0, 2).reshape(128, GH)))
        xt_feed = np.ascontiguousarray(np.concatenate(halves, axis=1)).astype(bf)
        in_maps.append({
            "xt": xt_feed,
            "wf": wf_feed,
            "wft": wft_feed,
            "sumrep": sumrep_np,
        })
    return in_maps


def kernel(x, W):
    global _cached
    x = np.asarray(x, dtype=np.float32)
    W = np.asarray(W, dtype=np.float32)
    if _cached is None:
        _cached = _build_program()
    nc = _cached
    in_maps = _prep_host(x, W)
    res = run_bass_kernel_spmd(nc, in_maps, list(range(NCORES)))
    out = np.concatenate(
        [res.results[c]["vout"].reshape(BC, J, E) for c in range(NCORES)], axis=0
    )
    return out.astype(np.float32)


if __name__ == "__main__":
    import sys
    sys.path.insert(0, "/root/problem")
    import reference as ref
    inputs = ref.setup_inputs()
    expected = np.asarray(ref.reference(**inputs))
    actual = kernel(np.asarray(inputs["x"]), np.asarray(inputs["W"]))
    err = np.abs(actual - expected)
    scale = np.abs(expected).max()
    print("absmax err:", err.max(), "scale:", scale, "rel:", err.max() / scale)
